# revision 15
# baseline (speedup 1.0000x reference)
"""Trainium2 Bass kernel for nn_GRUClassifier (B=64, T=512, E=256, H=512, 2-layer BiGRU + FC).

Strategy (8 cores, SPMD single program, zero control flow):
  - Cores form pairs (0,1),(2,3),(4,5),(6,7); each pair redundantly computes the
    full network. Even core runs the forward direction of both layers, odd core
    the backward direction (its `sentence` is time-reversed on the host, so the
    device program is identical).
  - Per core, per layer: a 512-step GRU chain in a stacked [128, 256] batch-major
    layout (partitions 0:64 = h[:, 0:256], 64:128 = h[:, 256:512]); gate
    pre-activations accumulate in PSUM via 2-way column-tiled matmuls.
  - To keep the PE dense (HAM warm) and shrink L1, the own-direction half of
    L1's input projection xg1own[t] = y0_own[t] @ W_ih1[own]^T (+ rz/xn biases)
    is computed DURING the L0 chain from the already-transposed hidden state,
    cast to bf16 and stored to DRAM; the L1 chain DMAs it back and injects it
    into PSUM with identity matmuls.
  - L0 stores transposed outputs y0T to DRAM; pairs exchange them with chunked
    2-rank AllGathers overlapped with the L0 chain. L1 reads the partner's
    (time-mirrored) y0T tiles via ONE indirect DMA per step whose offset table
    is a per-core host input - all rank/time asymmetry lives in input VALUES.
  - FC: each core computes its direction's half-product [64,10]; the host sums
    the pair and adds fc_b.
"""

import os
import numpy as np
import ml_dtypes

import concourse.bass as bass
import concourse.mybir as mybir
from concourse import bacc, tile
from concourse.bass_utils import run_bass_kernel_spmd

F32 = mybir.dt.float32
BF16 = mybir.dt.bfloat16
I32 = mybir.dt.int32

B = 64
H = 512
E = 256
V = 50000
NCORES = 8
AG_BLOCK = 32  # timesteps per AllGather chunk

bf = ml_dtypes.bfloat16

_BUILD_CACHE = {}


# ----------------------------------------------------------------------------
# host-side weight preparation
# ----------------------------------------------------------------------------

def _col_split_AB(WT):
    """WT: [Din, 1536] (cols = r|z|n each 512). Return (A, B) [Din, 768]:
    A = left halves of each gate's columns, B = right halves."""
    r, z, n = WT[:, 0:512], WT[:, 512:1024], WT[:, 1024:1536]
    A = np.concatenate([r[:, 0:256], z[:, 0:256], n[:, 0:256]], axis=1)
    Bm = np.concatenate([r[:, 256:512], z[:, 256:512], n[:, 256:512]], axis=1)
    return np.ascontiguousarray(A), np.ascontiguousarray(Bm)


def _bias_pair_rz(bih, bhh):
    """[2, 512] rows (left,right) of (bih+bhh) for r and z gates."""
    s = bih + bhh
    row0 = np.concatenate([s[0:256], s[512:768]])
    row1 = np.concatenate([s[256:512], s[768:1024]])
    return np.stack([row0, row1])


def _bias_pair_nx(bih, bhh):
    """[2, 512]: cols 0:256 = hn-side bias (bhh_n), 256:512 = xn-side (bih_n)."""
    row0 = np.concatenate([bhh[1024:1280], bih[1024:1280]])
    row1 = np.concatenate([bhh[1280:1536], bih[1280:1536]])
    return np.stack([row0, row1])


def make_core_inputs(core, T, sentence, emb, w_ih_l0, w_hh_l0, b_ih_l0, b_hh_l0,
                     w_ih_l1, w_hh_l1, b_ih_l1, b_hh_l1, fc_w, fc_b):
    d = core % 2  # 0 = forward, 1 = backward
    sent = np.asarray(sentence)[:, :T]
    if d == 1:
        sent = sent[:, ::-1]
    sent = sent.astype(np.int32)

    # gather index table [128, T//2]: tile s covers local steps 2s, 2s+1
    gidx = np.empty((128, T // 2), np.int32)
    gidx[0:64, :] = sent[:, 0::2]
    gidx[64:128, :] = sent[:, 1::2]

    wh0A, wh0B = _col_split_AB(np.asarray(w_hh_l0)[d].T)   # [512,768]
    wi0A, wi0B = _col_split_AB(np.asarray(w_ih_l0)[d].T)   # [256,768]
    wh1A, wh1B = _col_split_AB(np.asarray(w_hh_l1)[d].T)   # [512,768]
    WT1 = np.asarray(w_ih_l1)[d].T                         # [1024,1536]
    if d == 1:
        # odd core's own direction (rows 0:512) is the bwd half of y0
        WT1 = np.concatenate([WT1[512:1024], WT1[0:512]], axis=0)
    wg1A, wg1B = _col_split_AB(WT1[0:512])                 # own half   [512,768]
    wp1A, wp1B = _col_split_AB(WT1[512:1024])              # partner    [512,768]

    brz0 = _bias_pair_rz(np.asarray(b_ih_l0)[d], np.asarray(b_hh_l0)[d])
    bnx0 = _bias_pair_nx(np.asarray(b_ih_l0)[d], np.asarray(b_hh_l0)[d])
    brz1 = _bias_pair_rz(np.asarray(b_ih_l1)[d], np.asarray(b_hh_l1)[d])
    bnx1 = _bias_pair_nx(np.asarray(b_ih_l1)[d], np.asarray(b_hh_l1)[d])

    # L1 rz/xn bias rows folded into the xg1own ring: [2, 768]
    # row g = [ brz1[g] (512) | bih1_n half g (256) ]
    bt1 = np.empty((2, 768), np.float32)
    bt1[:, 0:512] = brz1
    bt1[:, 512:768] = bnx1[:, 256:512]

    # L1 hn bias rows [2, 256] (bhh1_n halves)
    bnh1 = np.ascontiguousarray(bnx1[:, 0:256])

    sel2 = np.zeros((2, 128), np.float32)
    sel2[0, 0:64] = 1.0
    sel2[1, 64:128] = 1.0

    fch = np.asarray(fc_w)[:, 512 * d:512 * d + 512].T    # [512, 10]
    fcw = np.ascontiguousarray(fch.reshape(4, 128, 10).transpose(1, 0, 2).reshape(128, 40))

    # partner y0T tile row-index table [128, 2T]
    # y0T_gath layout: [NB, 2, AG_BLOCK, 2, 128, 128]; flat rows of 128 elems:
    # row(blk, rank, pos, j, p) = (((blk*2+rank)*AG_BLOCK+pos)*2+j)*128 + p
    pr = 1 - d
    pidx = np.empty((128, 2 * T), np.int32)
    p = np.arange(128)
    for tau in range(T):
        tm = (T - 1) - tau
        blk, pos = tm // AG_BLOCK, tm % AG_BLOCK
        for j in (0, 1):
            base = (((blk * 2 + pr) * AG_BLOCK + pos) * 2 + j) * 128
            pidx[:, 2 * tau + j] = base + p

    return {
        "emb": np.asarray(emb, np.float32),
        "gidx": gidx,
        "pidx": pidx,
        "wh0A": wh0A.astype(bf), "wh0B": wh0B.astype(bf),
        "wi0A": wi0A.astype(bf), "wi0B": wi0B.astype(bf),
        "wh1A": wh1A.astype(bf), "wh1B": wh1B.astype(bf),
        "wg1A": wg1A.astype(bf), "wg1B": wg1B.astype(bf),
        "wp1A": wp1A.astype(bf), "wp1B": wp1B.astype(bf),
        "brz0": brz0.astype(bf), "bnx0": bnx0.astype(bf),
        "bnh1": bnh1.astype(bf),
        "bt1": bt1.astype(bf),
        "sel2": sel2.astype(bf),
        "identb": np.eye(128, dtype=np.float32).astype(bf),
        "fcw": fcw.astype(bf),
    }


# ----------------------------------------------------------------------------
# device program
# ----------------------------------------------------------------------------

def build_program(T):
    NB = T // AG_BLOCK
    nc = bacc.Bacc("TRN2", target_bir_lowering=False, debug=False,
                   enable_asserts=False, num_devices=NCORES)

    ein = lambda name, shape, dt: nc.dram_tensor(name, shape, dt, kind="ExternalInput")
    emb_d = ein("emb", [V, E], F32)
    gidx_d = ein("gidx", [128, T // 2], I32)
    pidx_d = ein("pidx", [128, 2 * T], I32)
    wh0A_d = ein("wh0A", [512, 768], BF16); wh0B_d = ein("wh0B", [512, 768], BF16)
    wi0A_d = ein("wi0A", [256, 768], BF16); wi0B_d = ein("wi0B", [256, 768], BF16)
    wh1A_d = ein("wh1A", [512, 768], BF16); wh1B_d = ein("wh1B", [512, 768], BF16)
    wg1A_d = ein("wg1A", [512, 768], BF16); wg1B_d = ein("wg1B", [512, 768], BF16)
    wp1A_d = ein("wp1A", [512, 768], BF16); wp1B_d = ein("wp1B", [512, 768], BF16)
    brz0_d = ein("brz0", [2, 512], BF16); bnx0_d = ein("bnx0", [2, 512], BF16)
    bnh1_d = ein("bnh1", [2, 256], BF16)
    bt1_d = ein("bt1", [2, 768], BF16)
    sel2_d = ein("sel2", [2, 128], BF16)
    identb_d = ein("identb", [128, 128], BF16)
    fcw_d = ein("fcw", [128, 40], BF16)

    out_d = nc.dram_tensor("out", [64, 10], F32, kind="ExternalOutput")

    y0T_self = nc.dram_tensor("y0Tself", [T, 2, 128, 128], BF16, kind="Internal")
    xg1_d = nc.dram_tensor("xg1own", [T, 128, 768], BF16, kind="Internal")
    y0T_gath = nc.dram_tensor("y0Tgath", [NB, 2, AG_BLOCK, 2, 128, 128], BF16,
                              kind="Internal")
    gath_rows = y0T_gath.ap().rearrange("a b c d p q -> (a b c d p) q")

    PAIRS = [[0, 1], [2, 3], [4, 5], [6, 7]]

    with tile.TileContext(nc) as tc:
        import contextlib
        ctx = contextlib.ExitStack()
        with ctx:
            cp = ctx.enter_context(tc.tile_pool(name="const", bufs=1))
            # constants into SBUF
            def load_w(dram, kchunks):
                t = cp.tile([128, kchunks * 768], BF16, tag=dram.name)
                for k in range(kchunks):
                    nc.sync.dma_start(out=t[:, 768 * k:768 * (k + 1)],
                                      in_=dram.ap()[128 * k:128 * (k + 1), :])
                return t
            wh0A = load_w(wh0A_d, 4); wh0B = load_w(wh0B_d, 4)
            wi0A = load_w(wi0A_d, 2); wi0B = load_w(wi0B_d, 2)
            wh1A = load_w(wh1A_d, 4); wh1B = load_w(wh1B_d, 4)
            wg1A = load_w(wg1A_d, 4); wg1B = load_w(wg1B_d, 4)
            wp1A = load_w(wp1A_d, 4); wp1B = load_w(wp1B_d, 4)

            def load_small(dram, shape, dt):
                t = cp.tile(list(shape), dt, tag=dram.name)
                nc.sync.dma_start(out=t[:, :], in_=dram.ap()[:, :])
                return t
            brz0 = load_small(brz0_d, (2, 512), BF16)
            bnx0 = load_small(bnx0_d, (2, 512), BF16)
            bnh1 = load_small(bnh1_d, (2, 256), BF16)
            bt1 = load_small(bt1_d, (2, 768), BF16)
            sel2 = load_small(sel2_d, (2, 128), BF16)
            identb = load_small(identb_d, (128, 128), BF16)
            fcw = load_small(fcw_d, (128, 40), BF16)
            gidx = load_small(gidx_d, (128, T // 2), I32)
            pidx = load_small(pidx_d, (128, 2 * T), I32)

            # pools  (PSUM: prz 2 + pnx 2 + pxg 2 + ptr 2 = 8 banks)
            prz_p = ctx.enter_context(tc.tile_pool(name="prz", bufs=2, space="PSUM"))
            pnx_p = ctx.enter_context(tc.tile_pool(name="pnx", bufs=2, space="PSUM"))
            pxg_p = ctx.enter_context(tc.tile_pool(name="pxg", bufs=1, space="PSUM"))
            ptr_p = ctx.enter_context(tc.tile_pool(name="ptr", bufs=2, space="PSUM"))
            xt_p = ctx.enter_context(tc.tile_pool(name="xt", bufs=2))
            xT_p = ctx.enter_context(tc.tile_pool(name="xT", bufs=2))
            h_p = ctx.enter_context(tc.tile_pool(name="h", bufs=2))
            hT_p = ctx.enter_context(tc.tile_pool(name="hT", bufs=3))
            sig_p = ctx.enter_context(tc.tile_pool(name="sig", bufs=2))
            t1_p = ctx.enter_context(tc.tile_pool(name="t1", bufs=2))
            t2_p = ctx.enter_context(tc.tile_pool(name="t2", bufs=2))
            nn_p = ctx.enter_context(tc.tile_pool(name="nn", bufs=2))
            dd_p = ctx.enter_context(tc.tile_pool(name="dd", bufs=2))
            ee_p = ctx.enter_context(tc.tile_pool(name="ee", bufs=2))
            xg_p = ctx.enter_context(tc.tile_pool(name="xg", bufs=3))
            rg_p = ctx.enter_context(tc.tile_pool(name="rg", bufs=3))
            pg_p = ctx.enter_context(tc.tile_pool(name="pg", bufs=3))
            yo_p = ctx.enter_context(tc.tile_pool(name="yo", bufs=3))
            fc_p = ctx.enter_context(tc.tile_pool(name="fc", bufs=1))

            MM = nc.tensor.matmul

            def lhsT_slice(hT, k):
                # k-chunk k of a [512,*] stationary held as ONE [128, 256]
                # tile (cols 0:128 = tile j0 (chunks 0/2), 128:256 = j1
                # (chunks 1/3); within a tile cols 0:64 = low chunk).
                j, c = k % 2, (k // 2) * 64
                return hT[:, 128 * j + c:128 * j + c + 64]

            # --- per-step matmul groups -------------------------------------
            # PSUM bank discipline: per bank exactly ONE start=True matmul,
            # issued first (clears the whole bank); later matmuls use
            # start=False (overwrite where has_written is clear, else add).

            def prep_l0_rz(xT, half):
                prz = prz_p.tile([128, 512], F32)
                MM(prz[:, :], sel2[:, :], brz0[:, :], start=True, stop=False,
                   skip_group_check=True)
                for k in range(2):
                    lt = xT[:, 128 * k + 64 * half:128 * k + 64 * half + 64]
                    c0 = 768 * k
                    last = k == 1
                    MM(prz[0:64, :], lt, wi0A[:, c0:c0 + 512], start=False,
                       stop=False, skip_group_check=True)
                    MM(prz[64:128, :], lt, wi0B[:, c0:c0 + 512], start=False,
                       stop=last, skip_group_check=True)
                return prz

            def prep_l0_nx(xT, half):
                pnx = pnx_p.tile([128, 512], F32)
                MM(pnx[:, :], sel2[:, :], bnx0[:, :], start=True, stop=False,
                   skip_group_check=True)
                for k in range(2):
                    lt = xT[:, 128 * k + 64 * half:128 * k + 64 * half + 64]
                    c0 = 768 * k
                    last = k == 1
                    MM(pnx[0:64, 256:512], lt, wi0A[:, c0 + 512:c0 + 768],
                       start=False, stop=False, skip_group_check=True)
                    MM(pnx[64:128, 256:512], lt, wi0B[:, c0 + 512:c0 + 768],
                       start=False, stop=last, skip_group_check=True)
                return pnx

            def prep_l1_rz(pg, rg, yo):
                prz = prz_p.tile([128, 512], F32)
                MM(prz[:, :], identb[:, :], rg[:, 0:512], start=True, stop=False,
                   skip_group_check=True)
                for k in range(4):
                    lt = lhsT_slice(pg, k)
                    c0 = 768 * k
                    MM(prz[0:64, :], lt, wp1A[:, c0:c0 + 512], start=False,
                       stop=False, skip_group_check=True)
                    MM(prz[64:128, :], lt, wp1B[:, c0:c0 + 512], start=False,
                       stop=False, skip_group_check=True)
                for k in (2, 3):
                    lt = lhsT_slice(yo, k)
                    c0 = 768 * k
                    last = k == 3
                    MM(prz[0:64, :], lt, wg1A[:, c0:c0 + 512], start=False,
                       stop=False, skip_group_check=True)
                    MM(prz[64:128, :], lt, wg1B[:, c0:c0 + 512], start=False,
                       stop=last, skip_group_check=True)
                return prz

            def prep_l1_nx(pg, rg, yo):
                pnx = pnx_p.tile([128, 512], F32)
                MM(pnx[:, 0:256], sel2[:, :], bnh1[:, :], start=True, stop=False,
                   skip_group_check=True)
                MM(pnx[:, 256:512], identb[:, :], rg[:, 512:768], start=False,
                   stop=False, skip_group_check=True)
                for k in range(4):
                    lt = lhsT_slice(pg, k)
                    c0 = 768 * k
                    MM(pnx[0:64, 256:512], lt, wp1A[:, c0 + 512:c0 + 768],
                       start=False, stop=False, skip_group_check=True)
                    MM(pnx[64:128, 256:512], lt, wp1B[:, c0 + 512:c0 + 768],
                       start=False, stop=False, skip_group_check=True)
                for k in (2, 3):
                    lt = lhsT_slice(yo, k)
                    c0 = 768 * k
                    last = k == 3
                    MM(pnx[0:64, 256:512], lt, wg1A[:, c0 + 512:c0 + 768],
                       start=False, stop=False, skip_group_check=True)
                    MM(pnx[64:128, 256:512], lt, wg1B[:, c0 + 512:c0 + 768],
                       start=False, stop=last, skip_group_check=True)
                return pnx

            def h_matmuls(layer, hT, prz, pnx, first):
                if first:
                    return  # h0 == 0: no contribution
                whA, whB = (wh0A, wh0B) if layer == 0 else (wh1A, wh1B)
                for k in range(4):
                    lt = lhsT_slice(hT, k)
                    c0 = 768 * k
                    last = k == 3
                    MM(prz[0:64, :], lt, whA[:, c0:c0 + 512], start=False,
                       stop=last, skip_group_check=True)
                    MM(prz[64:128, :], lt, whB[:, c0:c0 + 512], start=False,
                       stop=last, skip_group_check=True)
                for k in range(4):
                    lt = lhsT_slice(hT, k)
                    c0 = 768 * k
                    last = k == 3
                    MM(pnx[0:64, 0:256], lt, whA[:, c0 + 512:c0 + 768],
                       start=False, stop=last, skip_group_check=True)
                    MM(pnx[64:128, 0:256], lt, whB[:, c0 + 512:c0 + 768],
                       start=False, stop=last, skip_group_check=True)

            def gates(prz, pnx, h_prev):
                sig = sig_p.tile([128, 512], F32)
                nc.scalar.activation(sig[:, :], prz[:, :],
                                     mybir.ActivationFunctionType.Sigmoid)
                oz = dd_p.tile([128, 256], F32)
                nc.scalar.activation(oz[:, :], prz[:, 256:512],
                                     mybir.ActivationFunctionType.Sigmoid,
                                     scale=-1.0)
                if h_prev is not None:
                    zh = ee_p.tile([128, 256], F32)
                    nc.vector.tensor_tensor(out=zh[:, :], in0=sig[:, 256:512],
                                            in1=h_prev[:, :], op=mybir.AluOpType.mult)
                t1 = t1_p.tile([128, 256], F32)
                nc.vector.tensor_tensor(out=t1[:, :], in0=sig[:, 0:256],
                                        in1=pnx[:, 0:256], op=mybir.AluOpType.mult)
                t2 = t2_p.tile([128, 256], F32)
                nc.vector.tensor_tensor(out=t2[:, :], in0=t1[:, :],
                                        in1=pnx[:, 256:512], op=mybir.AluOpType.add)
                nn_t = nn_p.tile([128, 256], F32)
                nc.scalar.activation(nn_t[:, :], t2[:, :],
                                     mybir.ActivationFunctionType.Tanh)
                h_new = h_p.tile([128, 256], BF16)
                if h_prev is None:
                    nc.vector.tensor_tensor(out=h_new[:, :], in0=nn_t[:, :],
                                            in1=oz[:, :], op=mybir.AluOpType.mult)
                else:
                    nz = t1_p.tile([128, 256], F32, tag="nz")
                    nc.vector.tensor_tensor(out=nz[:, :], in0=nn_t[:, :],
                                            in1=oz[:, :], op=mybir.AluOpType.mult)
                    nc.vector.tensor_tensor(out=h_new[:, :], in0=nz[:, :],
                                            in1=zh[:, :], op=mybir.AluOpType.add)
                return h_new

            def transpose_h(h_new):
                pt = ptr_p.tile([128, 256], BF16, tag="pt")
                for j in (0, 1):
                    MM(pt[:, 128 * j:128 * (j + 1)],
                       h_new[:, 128 * j:128 * (j + 1)], identb[:, :],
                       is_transpose=True, start=(j == 0), stop=(j == 1),
                       skip_group_check=True)
                ht = hT_p.tile([128, 256], BF16)
                nc.vector.tensor_copy(out=ht[:, :], in_=pt[:, :])
                return ht

            def xg1own(tau, hT):
                """own-half L1 input projection for step tau -> DRAM ring."""
                pxg = pxg_p.tile([128, 768], F32)
                MM(pxg[:, 0:512], sel2[:, :], bt1[:, 0:512], start=True,
                   stop=False, skip_group_check=True)
                MM(pxg[:, 512:768], sel2[:, :], bt1[:, 512:768], start=True,
                   stop=False, skip_group_check=True)
                for k in range(2):
                    lt = lhsT_slice(hT, k)
                    c0 = 768 * k
                    last = k == 1
                    MM(pxg[0:64, 0:512], lt, wg1A[:, c0:c0 + 512],
                       start=False, stop=False, skip_group_check=True)
                    MM(pxg[64:128, 0:512], lt, wg1B[:, c0:c0 + 512],
                       start=False, stop=last, skip_group_check=True)
                    MM(pxg[0:64, 512:768], lt, wg1A[:, c0 + 512:c0 + 768],
                       start=False, stop=False, skip_group_check=True)
                    MM(pxg[64:128, 512:768], lt, wg1B[:, c0 + 512:c0 + 768],
                       start=False, stop=last, skip_group_check=True)
                xg = xg_p.tile([128, 768], BF16)
                nc.scalar.copy(out=xg[:, :], in_=pxg[:, :])
                nc.sync.dma_start(out=xg1_d.ap()[tau], in_=xg[:, :])

            # ---------------- L0 chain ----------------
            def gather_pair(s):
                xt = xt_p.tile([128, 256], F32)
                nc.gpsimd.indirect_dma_start(
                    out=xt[:, :], out_offset=None, in_=emb_d.ap(),
                    in_offset=bass.IndirectOffsetOnAxis(ap=gidx[:, s:s + 1], axis=0))
                xb = xt_p.tile([128, 256], BF16, tag="xb")
                nc.vector.tensor_copy(out=xb[:, :], in_=xt[:, :])
                px = ptr_p.tile([128, 256], BF16, tag="pt")
                for j in (0, 1):
                    MM(px[:, 128 * j:128 * (j + 1)],
                       xb[:, 128 * j:128 * (j + 1)], identb[:, :],
                       is_transpose=True, start=(j == 0), stop=(j == 1),
                       skip_group_check=True)
                xT = xT_p.tile([128, 256], BF16)
                nc.vector.tensor_copy(out=xT[:, :], in_=px[:, :])
                return xT

            h_prev, hT = None, None
            xT_cur = gather_pair(0)
            przs = {0: prep_l0_rz(xT_cur, 0), 1: prep_l0_rz(xT_cur, 1)}
            pnxs = {0: prep_l0_nx(xT_cur, 0), 1: prep_l0_nx(xT_cur, 1)}
            for tau in range(T):
                prz, pnx = przs.pop(tau), pnxs.pop(tau)
                h_matmuls(0, hT, prz, pnx, first=(tau == 0))
                h_new = gates(prz, pnx, h_prev)
                # fill PE while gates run: prep step tau+2, project xg1own
                # for the previous step (its hT is ready)
                if tau + 2 < T:
                    if (tau + 2) % 2 == 0:
                        xT_cur = gather_pair((tau + 2) // 2)
                    przs[tau + 2] = prep_l0_rz(xT_cur, (tau + 2) % 2)
                if hT is not None:
                    xg1own(tau - 1, hT)
                if tau + 2 < T:
                    pnxs[tau + 2] = prep_l0_nx(xT_cur, (tau + 2) % 2)
                hT = transpose_h(h_new)
                for j in (0, 1):
                    nc.sync.dma_start(out=y0T_self.ap()[tau, j],
                                      in_=hT[:, 128 * j:128 * (j + 1)])
                h_prev = h_new
                if (tau + 1) % AG_BLOCK == 0:
                    b = tau // AG_BLOCK
                    nc.gpsimd.collective_compute(
                        "AllGather", mybir.AluOpType.bypass,
                        replica_groups=PAIRS,
                        ins=[y0T_self.ap()[b * AG_BLOCK:(b + 1) * AG_BLOCK].opt()],
                        outs=[y0T_gath.ap()[b].opt()])
            xg1own(T - 1, hT)

            # ---------------- L1 chain ----------------
            def load_l1(tau):
                pg = pg_p.tile([128, 256], BF16)
                for j in (0, 1):
                    nc.gpsimd.indirect_dma_start(
                        out=pg[:, 128 * j:128 * (j + 1)], out_offset=None,
                        in_=gath_rows,
                        in_offset=bass.IndirectOffsetOnAxis(
                            ap=pidx[:, 2 * tau + j:2 * tau + j + 1], axis=0))
                rg = rg_p.tile([128, 768], BF16)
                nc.sync.dma_start(out=rg[:, :], in_=xg1_d.ap()[tau])
                yo = yo_p.tile([128, 256], BF16)
                for j in (0, 1):
                    nc.sync.dma_start(out=yo[:, 128 * j:128 * (j + 1)],
                                      in_=y0T_self.ap()[tau, j])
                return pg, rg, yo

            h_prev, hT = None, None
            rings = {0: load_l1(0), 1: load_l1(1)}
            przs = {t: prep_l1_rz(*rings[t]) for t in (0, 1)}
            pnxs = {t: prep_l1_nx(*rings[t]) for t in (0, 1)}
            for tau in range(T):
                prz, pnx = przs.pop(tau), pnxs.pop(tau)
                rings.pop(tau)
                h_matmuls(1, hT, prz, pnx, first=(tau == 0))
                h_new = gates(prz, pnx, h_prev)
                if tau + 2 < T:
                    rings[tau + 2] = load_l1(tau + 2)
                    przs[tau + 2] = prep_l1_rz(*rings[tau + 2])
                    pnxs[tau + 2] = prep_l1_nx(*rings[tau + 2])
                hT = transpose_h(h_new)
                h_prev = h_new

            # ---------------- FC ----------------
            pfc = prz_p.tile([64, 10], F32, tag="prz")
            for k in range(4):
                MM(pfc[:, :], lhsT_slice(hT, k), fcw[:, 10 * k:10 * (k + 1)],
                   start=(k == 0), stop=(k == 3), skip_group_check=True)
            fco = fc_p.tile([64, 10], F32)
            nc.vector.tensor_copy(out=fco[:, :], in_=pfc[:, :])
            nc.sync.dma_start(out=out_d.ap()[:, :], in_=fco[:, :])

    nc.compile()
    return nc


# ----------------------------------------------------------------------------
# entry point
# ----------------------------------------------------------------------------

def run(T, inputs, trace=False):
    key = T
    if key not in _BUILD_CACHE:
        _BUILD_CACHE[key] = build_program(T)
    nc = _BUILD_CACHE[key]
    in_maps = [make_core_inputs(c, T, **inputs) for c in range(NCORES)]
    res = run_bass_kernel_spmd(nc, in_maps, core_ids=list(range(NCORES)),
                               trace=trace)
    outs = res.results
    fc_b = np.asarray(inputs["fc_b"], np.float32)
    final = np.asarray(outs[0]["out"], np.float32) + np.asarray(outs[1]["out"], np.float32) + fc_b
    return final, res, outs


def kernel(sentence, emb, w_ih_l0, w_hh_l0, b_ih_l0, b_hh_l0,
           w_ih_l1, w_hh_l1, b_ih_l1, b_hh_l1, fc_w, fc_b):
    inputs = dict(sentence=sentence, emb=emb, w_ih_l0=w_ih_l0, w_hh_l0=w_hh_l0,
                  b_ih_l0=b_ih_l0, b_hh_l0=b_hh_l0, w_ih_l1=w_ih_l1,
                  w_hh_l1=w_hh_l1, b_ih_l1=b_ih_l1, b_hh_l1=b_hh_l1,
                  fc_w=fc_w, fc_b=fc_b)
    final, _, _ = run(np.asarray(sentence).shape[1], inputs)
    return final
